# revision 6
# baseline (speedup 1.0000x reference)
"""Trainium2 Bass kernel for nn_Attention_39934605918652.

res[b] = W0 @ x0[b] + sum_{n=1..N-1} W2 @ tanh(W1a @ x0[b] + W1b @ x[b,n])

Algebraic optimization: W2 is n-independent, so
    sum_n W2 @ tanh(...) = W2 @ (sum_n tanh(...))
leaving one [B,H]x[H,F] epilogue matmul.

Sharding: data-parallel over batch B=128 across 8 cores (16 batches/core),
weights replicated. No collectives.

v2 design (vs the 88us bf16 baseline): the three dominant costs were the
PE matmul stream (~55us bf16), the ACT tanh stream (129 per-batch calls,
~60us with per-call bubbles), and the DVE segmented reduce (~39us).
 - Main matmuls run in fp8e4m3 with MatmulPerfMode.DoubleRow: operands
   packed [128, ksub, cols] with contraction f = ksub*128 + p; each
   instruction contracts 2 ksubs (256 features).  W1b is pre-scaled by 32
   on host so its ~N(0, 1/1024) entries use e4m3's normal range; the tanh
   activation applies scale=1/32 to compensate.
 - The h0 bias is injected INTO PSUM with rank-1 fp16 matmuls
   (lhsT = h0row [1,128] at partition 0, rhs = ones [1,256]), so no
   separate elementwise bias pass exists and ACT needs no per-batch bias.
   h0 is computed batch-major ([16,1024] psum), cast/scaled to fp16 by one
   ACT copy, and flattened to partition 0 by one SBUF->SBUF DMA (engines
   can only address SBUF partition starts 0/32/64/96, so a [1, B*H] view
   is the only legal per-batch lhsT source).
 - tanh is ONE big ACT call per (h,q) psum tile [128,1024] (pad column
   included; excluded from the reduce), PSUM -> bf16 SBUF.
 - The segmented reduce_sum alternates between the DVE and the otherwise
   idle Pool (gpsimd) engine.
Measured end-to-end rel err vs a float64 oracle: ~2.5e-3 (fp8 per-element
noise averages down over the 255-term n-sum; harness gate is 2e-2).

Device layout (per core):
  xiT   [128, 4q*(4k*1024c)] fp8  col c = 256*b_in_q + n, f = k*128 + p
  x0T   [128, 4*BL]          fp16 host-packed f-chunks side by side
  w1bT  [128, 4k*1024h]      fp8  = 32*W1b.T packed like xiT
  w1aT  [512, 1024]          fp16 = W1a.T (rhs of batch-major h0 matmul)
  w2T   [1024, 512]          fp16 = W2.T
  w0T   [512, 512]           fp16 = W0.T
Output res [BL=16, 512] f32 per core; host concatenates.
"""

import os
import numpy as np
from contextlib import ExitStack

import concourse.bass as bass
import concourse.tile as tile
from concourse import bacc, mybir
from concourse.bass_utils import run_bass_kernel_spmd

N_CORES = 8
B, N, F, H = 128, 256, 512, 1024
BL = B // N_CORES          # 16 batches per core
NI = N - 1                 # 255 real columns per batch
NP = 256                   # padded columns per batch
NF = F // 128              # 4 f-chunks (= ksubs)
NH = H // 128              # 8 h-tiles
QUADS = BL // 4            # 4 batch-quads; per quad psum tile [128, 4*256]
W1B_SCALE = 32.0           # host pre-scale on W1b before fp8 quantization

F32 = mybir.dt.float32
BF16 = mybir.dt.bfloat16
F16 = mybir.dt.float16
F8 = mybir.dt.float8e4
DR = mybir.MatmulPerfMode.DoubleRow

# How many of the first psum tiles get their bias matmuls issued up front:
# they depend only on h0 (ready ~3us in), so they keep the PE busy/warm
# while xi still streams from HBM. Max useful = psum bufs.
N_PREBIAS = int(os.environ.get("KB_PREBIAS", "4"))


def _build_kernel():
    nc = bacc.Bacc(
        "TRN2", target_bir_lowering=False, debug=False, num_devices=N_CORES
    )

    xiT = nc.dram_tensor("xiT", [128, QUADS * NF * 1024], F8, kind="ExternalInput").ap()
    x0T = nc.dram_tensor("x0T", [128, NF * BL], F16, kind="ExternalInput").ap()
    w1bT = nc.dram_tensor("w1bT", [128, NF * H], F8, kind="ExternalInput").ap()
    w1aT = nc.dram_tensor("w1aT", [F, H], F16, kind="ExternalInput").ap()
    w2T = nc.dram_tensor("w2T", [H, F], F16, kind="ExternalInput").ap()
    w0T = nc.dram_tensor("w0T", [F, F], F16, kind="ExternalInput").ap()
    res = nc.dram_tensor("res", [BL, F], F32, kind="ExternalOutput").ap()

    with tile.TileContext(nc) as tc:
        with ExitStack() as ctx:
            _kernel_body(ctx, tc, xiT, x0T, w1bT, w1aT, w2T, w0T, res)

    nc.compile()
    return nc


def _kernel_body(ctx, tc, xiT, x0T, w1bT, w1aT, w2T, w0T, res):
    nc = tc.nc
    Tanh = mybir.ActivationFunctionType.Tanh
    Copy = mybir.ActivationFunctionType.Copy

    wpool = ctx.enter_context(tc.tile_pool(name="weights", bufs=1))

    def load(name, dram, rows, width, dt):
        tiles = []
        for c in range(rows // 128):
            t = wpool.tile([128, width], dt, tag=f"{name}_{c}", name=f"{name}_{c}")
            nc.sync.dma_start(t[:], dram[c * 128 : (c + 1) * 128, :])
            tiles.append(t)
        return tiles

    # DMA issue order = first-need order.
    x0_all = wpool.tile([128, NF * BL], F16, tag="x0", name="x0_all")
    nc.sync.dma_start(x0_all[:], x0T[:])
    x0_sb = [x0_all[:, f * BL : (f + 1) * BL] for f in range(NF)]
    w1a_sb = load("w1a", w1aT, F, H, F16)
    w1b_all = wpool.tile([128, NF * H], F8, tag="w1b", name="w1b_all")
    nc.sync.dma_start(w1b_all[:], w1bT[:])
    w1b_v = w1b_all[:].rearrange("p (k h) -> p k h", k=NF)
    xi_sb = []
    for q in range(QUADS):
        t = wpool.tile([128, NF * 1024], F8, tag=f"xi_{q}", name=f"xi_{q}")
        nc.sync.dma_start(t[:], xiT[:, q * NF * 1024 : (q + 1) * NF * 1024])
        xi_sb.append(t[:].rearrange("p (k c) -> p k c", k=NF))
    w2_sb = load("w2", w2T, H, F, F16)
    w0_sb = load("w0", w0T, F, F, F16)

    S_sb = [
        wpool.tile([128, BL], F16, tag=f"S_{h}", name=f"S_{h}")
        for h in range(NH)
    ]
    h0T_sb = wpool.tile([BL, H], F16, tag="h0T", name="h0T")
    h0f = wpool.tile([1, BL * H], F16, tag="h0f", name="h0f")
    # ones[0:255]=1, ones[255]=0: the bias matmul then leaves each batch's
    # pad column exactly 0 (xi pad col is 0 too), so tanh(0)=0 and the
    # reduce can cover the full 256-col block without masking.
    ones_sb = wpool.tile([1, NP], F16, tag="ones", name="ones")
    nc.gpsimd.memset(ones_sb[:], 1.0)
    nc.gpsimd.memset(ones_sb[0:1, NI:NP], 0.0)

    # One PSUM pool; shared tag so slots recycle. Slot = [128,1024] f32
    # = 2 banks; 4 bufs = 8 banks.
    ppool = ctx.enter_context(tc.tile_pool(name="ps", bufs=4, space="PSUM"))
    itpool = ctx.enter_context(tc.tile_pool(name="it", bufs=6))

    # ---- Phase 0: preload the tanh ACT table during the DMA lead-in
    # (first ACTIVATE otherwise pays the ~1.3us table load mid-kernel).
    tiny = wpool.tile([128, 1], F32, tag="tiny", name="tiny")
    nc.gpsimd.memset(tiny[:], 0.0)
    nc.scalar.activation(tiny[:], tiny[:], Tanh)

    # ---- Phase 1 (batch-major): h0T[b, h] = sum_f x0T[f, b] * W1aT[f, h],
    # then one ACT copy casts to fp16 with the 32x psum scale baked in, and
    # one SBUF->SBUF DMA flattens [16, 1024] -> [1, 16384] on partition 0
    # so per-(b, htile) rows are legal matmul lhsT slices.
    ph0 = ppool.tile([BL, H], F32, tag="ps", name="ph0")
    for hh in range(2):
        sl = slice(hh * 512, (hh + 1) * 512)
        for f in range(NF):
            nc.tensor.matmul(
                ph0[:, sl],
                x0_sb[f],
                w1a_sb[f][:, sl],
                start=(f == 0),
                stop=(f == NF - 1),
            )
    nc.scalar.activation(h0T_sb[:], ph0[:], Copy, scale=W1B_SCALE)
    nc.sync.dma_start(h0f[:], h0T_sb[:])

    def bias_mms(pb, h, q):
        # Four rank-1 fp16 matmuls add 32*h0[b, htile] to every column of
        # batch b's 256-col block. The PSUM start bit zeroes a whole 2KB
        # bank (512 f32 cols), so only the first matmul per bank sets it;
        # the second accumulates onto the bank's zeroed other half.
        for j in range(4):
            b = q * 4 + j
            nc.tensor.matmul(
                pb[:, j * NP : (j + 1) * NP],
                h0f[0:1, (b * NH + h) * 128 : (b * NH + h + 1) * 128],
                ones_sb[:],
                start=(j % 2 == 0),
                stop=False,
                skip_group_check=True,
            )

    def main_mms(pb, h, q):
        # fp8 DoubleRow: 2 ksub-pairs x 2 col-halves, each contracting 256
        # features over 512 psum columns.
        for bk in range(2):
            cols = slice(bk * 512, (bk + 1) * 512)
            for kp in range(2):
                nc.tensor.matmul(
                    pb[:, cols],
                    w1b_v[:, 2 * kp : 2 * kp + 2, h * 128 : (h + 1) * 128],
                    xi_sb[q][:, 2 * kp : 2 * kp + 2, cols],
                    start=False,
                    stop=(kp == 1),
                    perf_mode=DR,
                    skip_group_check=True,
                )

    def consume(h, q, pb, idx):
        # ACT: one big tanh over the whole psum tile (pad cols are exact
        # zeros). Pool: halving add folds cols [128:256) onto [0:128).
        # DVE: half-size segmented reduce. Splits the elementwise load
        # across three engines.
        it = itpool.tile([128, 4 * NP], BF16, tag="it", name=f"it_{h}_{q}")
        nc.scalar.activation(it[:], pb[:], Tanh, scale=1.0 / W1B_SCALE)
        v = it[:].rearrange("p (b n) -> p b n", b=4)
        half = itpool.tile([128, 4 * 128], BF16, tag="half", name=f"hf_{h}_{q}")
        hv = half[:].rearrange("p (b n) -> p b n", b=4)
        with nc.allow_low_precision(
            reason="S accumulated in 16-bit to feed the fp16 epilogue matmul"
        ):
            nc.gpsimd.tensor_add(hv, v[:, :, 0:128], v[:, :, 128:256])
            nc.vector.reduce_sum(
                S_sb[h][:, q * 4 : (q + 1) * 4],
                hv,
                axis=mybir.AxisListType.X,
            )

    # ---- Phase 2: interleaved bias + fp8 matmuls, tanh, segmented reduce.
    # The first N_PREBIAS tiles' psum tiles and bias matmuls are issued up
    # front: they only need h0 (ready early), keeping the PE warm while xi
    # streams in.
    order = [(h, 2 * w + i) for w in range(QUADS // 2) for h in range(NH) for i in range(2)]
    pbs = {}
    for h, q in order[:N_PREBIAS]:
        pb = pbs[(h, q)] = ppool.tile([128, 4 * NP], F32, tag="ps", name=f"pb_{h}_{q}")
        bias_mms(pb, h, q)
    for idx, (h, q) in enumerate(order):
        if (h, q) in pbs:
            pb = pbs.pop((h, q))
        else:
            pb = ppool.tile([128, 4 * NP], F32, tag="ps", name=f"pb_{h}_{q}")
            bias_mms(pb, h, q)
        main_mms(pb, h, q)
        consume(h, q, pb, idx)

    # ---- Phase 3: res[b, g] = sum_f x0T[f,b] W0T[f,g] + sum_h S[h,b] W2T[h,g]
    # b (=16) is the PE's M dim; 512 streams. W0 term first: no S dependency.
    po = ppool.tile([BL, F], F32, tag="ps", name="po")
    for f in range(NF):
        nc.tensor.matmul(
            po[:], x0_sb[f], w0_sb[f][:], start=(f == 0), stop=False
        )
    for h in range(NH):
        nc.tensor.matmul(
            po[:], S_sb[h][:], w2_sb[h][:], start=False, stop=(h == NH - 1)
        )
    rt = itpool.tile([BL, F], F32, tag="rt", name="rt")
    nc.vector.tensor_copy(rt[:], po[:])
    nc.sync.dma_start(res[:], rt[:])


_NC_CACHE = {}


def _get_nc():
    key = ("v2", N_PREBIAS)
    if key not in _NC_CACHE:
        _NC_CACHE[key] = _build_kernel()
    return _NC_CACHE[key]


def _make_in_maps(x, W1, W2, W0):
    import ml_dtypes
    f8 = ml_dtypes.float8_e4m3
    f16 = np.float16

    x = np.ascontiguousarray(np.asarray(x, dtype=np.float32))
    W1 = np.asarray(W1, dtype=np.float32)
    W2 = np.asarray(W2, dtype=np.float32)
    W0 = np.asarray(W0, dtype=np.float32)

    # [p, k, h] = 32 * W1b[h, k*128+p]
    w1bT = np.ascontiguousarray(
        (W1[:, F:].T * W1B_SCALE).reshape(NF, 128, H).transpose(1, 0, 2).reshape(128, NF * H)
    ).astype(f8)
    w1aT = np.ascontiguousarray(W1[:, :F].T).astype(f16)   # [F, H]
    w2T = np.ascontiguousarray(W2.T).astype(f16)           # [H, F]
    w0T = np.ascontiguousarray(W0.T).astype(f16)           # [F, F]

    in_maps = []
    for i in range(N_CORES):
        xc = x[i * BL : (i + 1) * BL]               # [BL, N, F]
        # packed [128, NF*BL]: row p, block f holds x0T[f*128+p, :]
        x0Tc = np.ascontiguousarray(
            xc[:, 0, :].T.reshape(NF, 128, BL).transpose(1, 0, 2).reshape(128, NF * BL)
        ).astype(f16)
        pad = np.zeros((BL, NP, F), dtype=np.float32)
        pad[:, :NI, :] = xc[:, 1:, :]
        xiT_full = pad.reshape(BL * NP, F).T        # [512, 4096], col = 256*b + n
        # [p, q, k, c] = xiT_full[k*128+p, q*1024+c]
        xiTc = np.ascontiguousarray(
            xiT_full.reshape(NF, 128, QUADS, 1024)
            .transpose(1, 2, 0, 3)
            .reshape(128, QUADS * NF * 1024)
        ).astype(f8)
        in_maps.append(
            {
                "xiT": xiTc,
                "x0T": x0Tc,
                "w1bT": w1bT,
                "w1aT": w1aT,
                "w2T": w2T,
                "w0T": w0T,
            }
        )
    return in_maps


def _gather(results):
    out = np.empty((B, F), dtype=np.float32)
    for i in range(N_CORES):
        out[i * BL : (i + 1) * BL] = results[i]["res"]
    return out


def kernel(x, W1, W2, W0):
    nc = _get_nc()
    in_maps = _make_in_maps(x, W1, W2, W0)
    res = run_bass_kernel_spmd(nc, in_maps, list(range(N_CORES)))
    return _gather(res.results)


def kernel_profiled(x, W1, W2, W0, **trace_kwargs):
    """Like kernel() but with NTFF profiling; returns (out, exec_time_ns)."""
    nc = _get_nc()
    in_maps = _make_in_maps(x, W1, W2, W0)
    res = run_bass_kernel_spmd(
        nc, in_maps, list(range(N_CORES)), trace=True, **trace_kwargs
    )
    return _gather(res.results), res.exec_time_ns


# revision 11
# speedup vs baseline: 1.0785x; 1.0785x over previous
"""Trainium2 Bass kernel for nn_Attention_39934605918652.

res[b] = W0 @ x0[b] + sum_{n=1..N-1} W2 @ tanh(W1a @ x0[b] + W1b @ x[b,n])

Algebraic optimization: W2 is n-independent, so
    sum_n W2 @ tanh(...) = W2 @ (sum_n tanh(...))
leaving one [B,H]x[H,F] epilogue matmul.

Sharding: data-parallel over batch B=128 across 8 cores (16 batches/core),
weights replicated. No collectives.

v2 design (vs the 88us bf16 baseline): the three dominant costs were the
PE matmul stream (~55us bf16), the ACT tanh stream (129 per-batch calls,
~60us with per-call bubbles), and the DVE segmented reduce (~39us).
 - Main matmuls run in fp8e4m3 with MatmulPerfMode.DoubleRow: operands
   packed [128, ksub, cols] with contraction f = ksub*128 + p; each
   instruction contracts 2 ksubs (256 features).  W1b is pre-scaled by 32
   on host so its ~N(0, 1/1024) entries use e4m3's normal range; the tanh
   activation applies scale=1/32 to compensate.
 - The h0 bias is injected INTO PSUM with rank-1 fp16 matmuls
   (lhsT = h0row [1,128] at partition 0, rhs = ones [1,256]), so no
   separate elementwise bias pass exists and ACT needs no per-batch bias.
   h0 is computed batch-major ([16,1024] psum), cast/scaled to fp16 by one
   ACT copy, and flattened to partition 0 by one SBUF->SBUF DMA (engines
   can only address SBUF partition starts 0/32/64/96, so a [1, B*H] view
   is the only legal per-batch lhsT source).
 - tanh is ONE big ACT call per (h,q) psum tile [128,1024] (pad column
   included; excluded from the reduce), PSUM -> bf16 SBUF.
 - The segmented reduce_sum alternates between the DVE and the otherwise
   idle Pool (gpsimd) engine.
Measured end-to-end rel err vs a float64 oracle: ~2.5e-3 (fp8 per-element
noise averages down over the 255-term n-sum; harness gate is 2e-2).

Device layout (per core):
  xiT   [128, 4q*(4k*1024c)] fp8  col c = 256*b_in_q + n, f = k*128 + p
  x0T   [128, 4*BL]          fp16 host-packed f-chunks side by side
  w1bT  [128, 4k*1024h]      fp8  = 32*W1b.T packed like xiT
  w1aT  [512, 1024]          fp16 = W1a.T (rhs of batch-major h0 matmul)
  w2T   [1024, 512]          fp16 = W2.T
  w0T   [512, 512]           fp16 = W0.T
Output res [BL=16, 512] f32 per core; host concatenates.
"""

import os
import numpy as np
from contextlib import ExitStack

import concourse.bass as bass
import concourse.tile as tile
from concourse import bacc, mybir
from concourse.bass_utils import run_bass_kernel_spmd

N_CORES = 8
B, N, F, H = 128, 256, 512, 1024
BL = B // N_CORES          # 16 batches per core
NI = N - 1                 # 255 real columns per batch
NP = 256                   # padded columns per batch
NF = F // 128              # 4 f-chunks (= ksubs)
NH = H // 128              # 8 h-tiles
QUADS = BL // 4            # 4 batch-quads; per quad psum tile [128, 4*256]
W1B_SCALE = 32.0           # host pre-scale on W1b before fp8 quantization

F32 = mybir.dt.float32
BF16 = mybir.dt.bfloat16
F16 = mybir.dt.float16
F8 = mybir.dt.float8e4
DR = mybir.MatmulPerfMode.DoubleRow

# How many of the first psum tiles get their bias matmuls issued up front:
# they depend only on h0 (ready ~3us in), so they keep the PE busy/warm
# while xi still streams from HBM. Max useful = psum bufs.
N_PREBIAS = int(os.environ.get("KB_PREBIAS", "4"))
# Dummy fp32 matmuls on zeros (no DMA dependency) issued first: HAM only
# grants the PE full clock after ~13us of sustained activity, and the
# DMA lead-in otherwise leaves it idle until ~16us. Each is ~427ns warm /
# ~850ns at the half-rate p-state.
N_WARM = int(os.environ.get("KB_WARM", "12"))


def _build_kernel():
    nc = bacc.Bacc(
        "TRN2", target_bir_lowering=False, debug=False, num_devices=N_CORES
    )

    xiT = nc.dram_tensor("xiT", [128, QUADS * NF * 1024], F8, kind="ExternalInput").ap()
    x0T = nc.dram_tensor("x0T", [128, NF * BL], F16, kind="ExternalInput").ap()
    w1bT = nc.dram_tensor("w1bT", [128, NF * H], F8, kind="ExternalInput").ap()
    w1aT = nc.dram_tensor("w1aT", [F, H], F16, kind="ExternalInput").ap()
    w2T = nc.dram_tensor("w2T", [H, F], F16, kind="ExternalInput").ap()
    w0T = nc.dram_tensor("w0T", [F, F], F16, kind="ExternalInput").ap()
    res = nc.dram_tensor("res", [BL, F], F32, kind="ExternalOutput").ap()

    with tile.TileContext(nc) as tc:
        with ExitStack() as ctx:
            _kernel_body(ctx, tc, xiT, x0T, w1bT, w1aT, w2T, w0T, res)

    nc.compile()
    return nc


def _kernel_body(ctx, tc, xiT, x0T, w1bT, w1aT, w2T, w0T, res):
    nc = tc.nc
    Tanh = mybir.ActivationFunctionType.Tanh
    Copy = mybir.ActivationFunctionType.Copy

    wpool = ctx.enter_context(tc.tile_pool(name="weights", bufs=1))

    def load(name, dram, rows, width, dt):
        tiles = []
        for c in range(rows // 128):
            t = wpool.tile([128, width], dt, tag=f"{name}_{c}", name=f"{name}_{c}")
            nc.sync.dma_start(t[:], dram[c * 128 : (c + 1) * 128, :])
            tiles.append(t)
        return tiles

    # DMA issue order = first-need order.
    x0_all = wpool.tile([128, NF * BL], F16, tag="x0", name="x0_all")
    nc.sync.dma_start(x0_all[:], x0T[:])
    x0_sb = [x0_all[:, f * BL : (f + 1) * BL] for f in range(NF)]
    w1a_sb = load("w1a", w1aT, F, H, F16)
    w1b_all = wpool.tile([128, NF * H], F8, tag="w1b", name="w1b_all")
    nc.sync.dma_start(w1b_all[:], w1bT[:])
    w1b_v = w1b_all[:].rearrange("p (k h) -> p k h", k=NF)
    xi_sb = []
    for q in range(QUADS):
        t = wpool.tile([128, NF * 1024], F8, tag=f"xi_{q}", name=f"xi_{q}")
        nc.sync.dma_start(t[:], xiT[:, q * NF * 1024 : (q + 1) * NF * 1024])
        xi_sb.append(t[:].rearrange("p (k c) -> p k c", k=NF))
    w2_sb = load("w2", w2T, H, F, F16)
    w0_sb = load("w0", w0T, F, F, F16)

    S_sb = [
        wpool.tile([128, BL], F16, tag=f"S_{h}", name=f"S_{h}")
        for h in range(NH)
    ]
    h0T_sb = wpool.tile([BL, H], F16, tag="h0T", name="h0T")
    h0f = wpool.tile([1, BL * H], F16, tag="h0f", name="h0f")
    # ones[0:255]=1, ones[255]=0: the bias matmul then leaves each batch's
    # pad column exactly 0 (xi pad col is 0 too), so tanh(0)=0 and the
    # reduce can cover the full 256-col block without masking.
    ones_sb = wpool.tile([1, NP], F16, tag="ones", name="ones")
    nc.gpsimd.memset(ones_sb[:], 1.0)
    nc.gpsimd.memset(ones_sb[0:1, NI:NP], 0.0)

    # One PSUM pool; shared tag so slots recycle. Slot = [128,1024] f32
    # = 2 banks; 4 bufs = 8 banks.
    ppool = ctx.enter_context(tc.tile_pool(name="ps", bufs=4, space="PSUM"))
    itpool = ctx.enter_context(tc.tile_pool(name="it", bufs=6))

    # ---- Phase 0: preload the tanh ACT table during the DMA lead-in
    # (first ACTIVATE otherwise pays the ~1.3us table load mid-kernel).
    tiny = wpool.tile([128, 1], F32, tag="tiny", name="tiny")
    nc.gpsimd.memset(tiny[:], 0.0)
    nc.scalar.activation(tiny[:], tiny[:], Tanh)

    # ---- Phase 0b: PE warm-up during the DMA lead-in (see N_WARM).
    if N_WARM:
        wz = wpool.tile([128, 256], F32, tag="warmz", name="warmz")
        nc.gpsimd.memset(wz[:], 0.0)
        pw = ppool.tile([128, 256], F32, tag="ps", name="pwarm")
        for _ in range(N_WARM):
            nc.tensor.matmul(pw[:], wz[:, :128], wz[:], start=True, stop=True)

    # ---- Phase 1 (batch-major): h0T[b, h] = sum_f x0T[f, b] * W1aT[f, h],
    # then one ACT copy casts to fp16 with the 32x psum scale baked in, and
    # one SBUF->SBUF DMA flattens [16, 1024] -> [1, 16384] on partition 0
    # so per-(b, htile) rows are legal matmul lhsT slices.
    ph0 = ppool.tile([BL, H], F32, tag="ps", name="ph0")
    for hh in range(2):
        sl = slice(hh * 512, (hh + 1) * 512)
        for f in range(NF):
            nc.tensor.matmul(
                ph0[:, sl],
                x0_sb[f],
                w1a_sb[f][:, sl],
                start=(f == 0),
                stop=(f == NF - 1),
            )
    # The flatten DMA goes on the scalar queue: on the sync queue it would
    # sit behind the bulk input loads and stall every bias matmul ~14us.
    nc.scalar.activation(h0T_sb[:], ph0[:], Copy, scale=W1B_SCALE)
    nc.scalar.dma_start(h0f[:], h0T_sb[:])

    def bias_mms(pb, h, q):
        # Four rank-1 fp16 matmuls add 32*h0[b, htile] to every column of
        # batch b's 256-col block. The PSUM start bit zeroes a whole 2KB
        # bank (512 f32 cols), so only the first matmul per bank sets it;
        # the second accumulates onto the bank's zeroed other half.
        for j in range(4):
            b = q * 4 + j
            nc.tensor.matmul(
                pb[:, j * NP : (j + 1) * NP],
                h0f[0:1, (b * NH + h) * 128 : (b * NH + h + 1) * 128],
                ones_sb[:],
                start=(j % 2 == 0),
                stop=False,
                skip_group_check=True,
            )

    def main_mms(pb, h, q):
        # fp8 DoubleRow: 2 ksub-pairs x 2 col-halves, each contracting 256
        # features over 512 psum columns.
        for bk in range(2):
            cols = slice(bk * 512, (bk + 1) * 512)
            for kp in range(2):
                nc.tensor.matmul(
                    pb[:, cols],
                    w1b_v[:, 2 * kp : 2 * kp + 2, h * 128 : (h + 1) * 128],
                    xi_sb[q][:, 2 * kp : 2 * kp + 2, cols],
                    start=False,
                    stop=(kp == 1),
                    perf_mode=DR,
                    skip_group_check=True,
                )

    def consume(h, q, pb, idx):
        # ACT: one big tanh over the whole psum tile (pad cols are exact
        # zeros). Pool: halving add folds cols [128:256) onto [0:128).
        # DVE: half-size segmented reduce. Splits the elementwise load
        # across three engines.
        it = itpool.tile([128, 4 * NP], BF16, tag="it", name=f"it_{h}_{q}")
        nc.scalar.activation(it[:], pb[:], Tanh, scale=1.0 / W1B_SCALE)
        v = it[:].rearrange("p (b n) -> p b n", b=4)
        half = itpool.tile([128, 4 * 128], BF16, tag="half", name=f"hf_{h}_{q}")
        hv = half[:].rearrange("p (b n) -> p b n", b=4)
        # Pool (gpsimd) measures ~2.25ns/elem on TENSOR_TENSOR vs DVE's
        # 2x_1p mode at ~0.5ns/elem, so alternate adds between them to
        # balance; reduce_sum is DVE-only for the free axis.
        add_eng = nc.gpsimd if idx % 2 == 0 else nc.vector
        with nc.allow_low_precision(
            reason="S accumulated in 16-bit to feed the fp16 epilogue matmul"
        ):
            add_eng.tensor_add(hv, v[:, :, 0:128], v[:, :, 128:256])
            nc.vector.reduce_sum(
                S_sb[h][:, q * 4 : (q + 1) * 4],
                hv,
                axis=mybir.AxisListType.X,
            )

    # ---- Phase 2: interleaved bias + fp8 matmuls, tanh, segmented reduce.
    # The first N_PREBIAS tiles' psum tiles and bias matmuls are issued up
    # front: they only need h0 (ready early), keeping the PE warm while xi
    # streams in.
    order = [(h, 2 * w + i) for w in range(QUADS // 2) for h in range(NH) for i in range(2)]
    pbs = {}
    for h, q in order[:N_PREBIAS]:
        pb = pbs[(h, q)] = ppool.tile([128, 4 * NP], F32, tag="ps", name=f"pb_{h}_{q}")
        bias_mms(pb, h, q)
    for idx, (h, q) in enumerate(order):
        if (h, q) in pbs:
            pb = pbs.pop((h, q))
        else:
            pb = ppool.tile([128, 4 * NP], F32, tag="ps", name=f"pb_{h}_{q}")
            bias_mms(pb, h, q)
        main_mms(pb, h, q)
        consume(h, q, pb, idx)

    # ---- Phase 3: res[b, g] = sum_f x0T[f,b] W0T[f,g] + sum_h S[h,b] W2T[h,g]
    # b (=16) is the PE's M dim; 512 streams. W0 term first: no S dependency.
    po = ppool.tile([BL, F], F32, tag="ps", name="po")
    for f in range(NF):
        nc.tensor.matmul(
            po[:], x0_sb[f], w0_sb[f][:], start=(f == 0), stop=False
        )
    for h in range(NH):
        nc.tensor.matmul(
            po[:], S_sb[h][:], w2_sb[h][:], start=False, stop=(h == NH - 1)
        )
    rt = itpool.tile([BL, F], F32, tag="rt", name="rt")
    nc.vector.tensor_copy(rt[:], po[:])
    nc.sync.dma_start(res[:], rt[:])


_NC_CACHE = {}


def _get_nc():
    key = ("v3", N_PREBIAS, N_WARM)
    if key not in _NC_CACHE:
        _NC_CACHE[key] = _build_kernel()
    return _NC_CACHE[key]


def _make_in_maps(x, W1, W2, W0):
    import ml_dtypes
    f8 = ml_dtypes.float8_e4m3
    f16 = np.float16

    x = np.ascontiguousarray(np.asarray(x, dtype=np.float32))
    W1 = np.asarray(W1, dtype=np.float32)
    W2 = np.asarray(W2, dtype=np.float32)
    W0 = np.asarray(W0, dtype=np.float32)

    # [p, k, h] = 32 * W1b[h, k*128+p]
    w1bT = np.ascontiguousarray(
        (W1[:, F:].T * W1B_SCALE).reshape(NF, 128, H).transpose(1, 0, 2).reshape(128, NF * H)
    ).astype(f8)
    w1aT = np.ascontiguousarray(W1[:, :F].T).astype(f16)   # [F, H]
    w2T = np.ascontiguousarray(W2.T).astype(f16)           # [H, F]
    w0T = np.ascontiguousarray(W0.T).astype(f16)           # [F, F]

    in_maps = []
    for i in range(N_CORES):
        xc = x[i * BL : (i + 1) * BL]               # [BL, N, F]
        # packed [128, NF*BL]: row p, block f holds x0T[f*128+p, :]
        x0Tc = np.ascontiguousarray(
            xc[:, 0, :].T.reshape(NF, 128, BL).transpose(1, 0, 2).reshape(128, NF * BL)
        ).astype(f16)
        pad = np.zeros((BL, NP, F), dtype=np.float32)
        pad[:, :NI, :] = xc[:, 1:, :]
        xiT_full = pad.reshape(BL * NP, F).T        # [512, 4096], col = 256*b + n
        # [p, q, k, c] = xiT_full[k*128+p, q*1024+c]
        xiTc = np.ascontiguousarray(
            xiT_full.reshape(NF, 128, QUADS, 1024)
            .transpose(1, 2, 0, 3)
            .reshape(128, QUADS * NF * 1024)
        ).astype(f8)
        in_maps.append(
            {
                "xiT": xiTc,
                "x0T": x0Tc,
                "w1bT": w1bT,
                "w1aT": w1aT,
                "w2T": w2T,
                "w0T": w0T,
            }
        )
    return in_maps


def _gather(results):
    out = np.empty((B, F), dtype=np.float32)
    for i in range(N_CORES):
        out[i * BL : (i + 1) * BL] = results[i]["res"]
    return out


def kernel(x, W1, W2, W0):
    nc = _get_nc()
    in_maps = _make_in_maps(x, W1, W2, W0)
    res = run_bass_kernel_spmd(nc, in_maps, list(range(N_CORES)))
    return _gather(res.results)


def kernel_profiled(x, W1, W2, W0, **trace_kwargs):
    """Like kernel() but with NTFF profiling; returns (out, exec_time_ns)."""
    nc = _get_nc()
    in_maps = _make_in_maps(x, W1, W2, W0)
    res = run_bass_kernel_spmd(
        nc, in_maps, list(range(N_CORES)), trace=True, **trace_kwargs
    )
    return _gather(res.results), res.exec_time_ns


# revision 15
# speedup vs baseline: 1.0942x; 1.0146x over previous
"""Trainium2 Bass kernel for nn_Attention_39934605918652.

res[b] = W0 @ x0[b] + sum_{n=1..N-1} W2 @ tanh(W1a @ x0[b] + W1b @ x[b,n])

Algebraic optimization: W2 is n-independent, so
    sum_n W2 @ tanh(...) = W2 @ (sum_n tanh(...))
leaving one [B,H]x[H,F] epilogue matmul.

Sharding: data-parallel over batch B=128 across 8 cores (16 batches/core),
weights replicated. No collectives.

v2 design (vs the 88us bf16 baseline): the three dominant costs were the
PE matmul stream (~55us bf16), the ACT tanh stream (129 per-batch calls,
~60us with per-call bubbles), and the DVE segmented reduce (~39us).
 - Main matmuls run in fp8e4m3 with MatmulPerfMode.DoubleRow: operands
   packed [128, ksub, cols] with contraction f = ksub*128 + p; each
   instruction contracts 2 ksubs (256 features).  W1b is pre-scaled by 32
   on host so its ~N(0, 1/1024) entries use e4m3's normal range; the tanh
   activation applies scale=1/32 to compensate.
 - The h0 bias is injected INTO PSUM with rank-1 fp16 matmuls
   (lhsT = h0row [1,128] at partition 0, rhs = ones [1,256]), so no
   separate elementwise bias pass exists and ACT needs no per-batch bias.
   h0 is computed batch-major ([16,1024] psum), cast/scaled to fp16 by one
   ACT copy, and flattened to partition 0 by one SBUF->SBUF DMA (engines
   can only address SBUF partition starts 0/32/64/96, so a [1, B*H] view
   is the only legal per-batch lhsT source).
 - tanh is ONE big ACT call per (h,q) psum tile [128,1024] (pad column
   included; excluded from the reduce), PSUM -> bf16 SBUF.
 - The segmented reduce_sum alternates between the DVE and the otherwise
   idle Pool (gpsimd) engine.
Measured end-to-end rel err vs a float64 oracle: ~2.5e-3 (fp8 per-element
noise averages down over the 255-term n-sum; harness gate is 2e-2).

Device layout (per core):
  xiT   [128, 4q*(4k*1024c)] fp8  col c = 256*b_in_q + n, f = k*128 + p
  x0T   [128, 4*BL]          fp16 host-packed f-chunks side by side
  w1bT  [128, 4k*1024h]      fp8  = 32*W1b.T packed like xiT
  w1aT  [512, 1024]          fp16 = W1a.T (rhs of batch-major h0 matmul)
  w2T   [1024, 512]          fp16 = W2.T
  w0T   [512, 512]           fp16 = W0.T
Output res [BL=16, 512] f32 per core; host concatenates.
"""

import os
import numpy as np
from contextlib import ExitStack

import concourse.bass as bass
import concourse.tile as tile
from concourse import bacc, mybir
from concourse.bass_utils import run_bass_kernel_spmd

N_CORES = 8
B, N, F, H = 128, 256, 512, 1024
BL = B // N_CORES          # 16 batches per core
NI = N - 1                 # 255 real columns per batch
NP = 256                   # padded columns per batch
NF = F // 128              # 4 f-chunks (= ksubs)
NH = H // 128              # 8 h-tiles
QUADS = BL // 4            # 4 batch-quads; per quad psum tile [128, 4*256]
W1B_SCALE = 32.0           # host pre-scale on W1b before fp8 quantization

F32 = mybir.dt.float32
BF16 = mybir.dt.bfloat16
F16 = mybir.dt.float16
F8 = mybir.dt.float8e4
DR = mybir.MatmulPerfMode.DoubleRow

# How many of the first psum tiles get their bias matmuls issued up front:
# they depend only on h0 (ready ~3us in), so they keep the PE busy/warm
# while xi still streams from HBM. Max useful = psum bufs.
N_PREBIAS = int(os.environ.get("KB_PREBIAS", "2"))
# Dummy fp32 matmuls on zeros (no DMA dependency): HAM only grants the PE
# full clock after ~6us of sustained activity and re-throttles to half
# rate after a multi-us idle gap, so the lead-in gaps are bridged with
# busywork. N_WARM runs before phase 1 (PE idle 6->11us while w1a
# streams); N_WARM2 runs after the prebias block (PE idle ~14->17us
# while xi0 streams). Each is ~427ns warm / ~850ns at half rate.
N_WARM = int(os.environ.get("KB_WARM", "6"))
N_WARM2 = int(os.environ.get("KB_WARM2", "4"))


def _build_kernel():
    nc = bacc.Bacc(
        "TRN2", target_bir_lowering=False, debug=False, num_devices=N_CORES
    )

    xiT = nc.dram_tensor("xiT", [128, QUADS * NF * 1024], F8, kind="ExternalInput").ap()
    x0T = nc.dram_tensor("x0T", [128, NF * BL], F16, kind="ExternalInput").ap()
    w1bT = nc.dram_tensor("w1bT", [128, NF * H], F8, kind="ExternalInput").ap()
    w1aT = nc.dram_tensor("w1aT", [F, H], F16, kind="ExternalInput").ap()
    w2T = nc.dram_tensor("w2T", [H, F], F16, kind="ExternalInput").ap()
    w0T = nc.dram_tensor("w0T", [F, F], F16, kind="ExternalInput").ap()
    res = nc.dram_tensor("res", [BL, F], F32, kind="ExternalOutput").ap()

    with tile.TileContext(nc) as tc:
        with ExitStack() as ctx:
            _kernel_body(ctx, tc, xiT, x0T, w1bT, w1aT, w2T, w0T, res)

    nc.compile()
    return nc


def _kernel_body(ctx, tc, xiT, x0T, w1bT, w1aT, w2T, w0T, res):
    nc = tc.nc
    Tanh = mybir.ActivationFunctionType.Tanh
    Copy = mybir.ActivationFunctionType.Copy

    wpool = ctx.enter_context(tc.tile_pool(name="weights", bufs=1))

    def load(name, dram, rows, width, dt):
        tiles = []
        for c in range(rows // 128):
            t = wpool.tile([128, width], dt, tag=f"{name}_{c}", name=f"{name}_{c}")
            nc.sync.dma_start(t[:], dram[c * 128 : (c + 1) * 128, :])
            tiles.append(t)
        return tiles

    # Warm the scalar engine's DMA queue with a tiny transfer: the first
    # use of a DGE queue costs ~6us, which would otherwise land on the
    # h0 flatten DMA in the middle of the critical path.
    dq = wpool.tile([1, 128], F16, tag="dq", name="dq")
    nc.gpsimd.memset(dq[:], 0.0)
    nc.scalar.dma_start(dq[0:1, 64:128], dq[0:1, 0:64])

    # DMA issue order = first-need order. w1a arrives split by H-halves so
    # phase 1 (and the bias matmuls that depend on it) can start after
    # half the weight load.
    x0_all = wpool.tile([128, NF * BL], F16, tag="x0", name="x0_all")
    nc.sync.dma_start(x0_all[:], x0T[:])
    x0_sb = [x0_all[:, f * BL : (f + 1) * BL] for f in range(NF)]
    w1a_sb = [[None] * NF for _ in range(2)]
    for hh in range(2):
        for f in range(NF):
            t = wpool.tile([128, 512], F16, tag=f"w1a_{hh}_{f}", name=f"w1a_{hh}_{f}")
            nc.sync.dma_start(
                t[:], w1aT[f * 128 : (f + 1) * 128, hh * 512 : (hh + 1) * 512]
            )
            w1a_sb[hh][f] = t
    w1b_all = wpool.tile([128, NF * H], F8, tag="w1b", name="w1b_all")
    nc.sync.dma_start(w1b_all[:], w1bT[:])
    w1b_v = w1b_all[:].rearrange("p (k h) -> p k h", k=NF)
    xi_sb = []
    for q in range(QUADS):
        t = wpool.tile([128, NF * 1024], F8, tag=f"xi_{q}", name=f"xi_{q}")
        nc.sync.dma_start(t[:], xiT[:, q * NF * 1024 : (q + 1) * NF * 1024])
        xi_sb.append(t[:].rearrange("p (k c) -> p k c", k=NF))
    w2_sb = load("w2", w2T, H, F, F16)
    w0_sb = load("w0", w0T, F, F, F16)

    S_sb = [
        wpool.tile([128, BL], F16, tag=f"S_{h}", name=f"S_{h}")
        for h in range(NH)
    ]
    h0T_sb = wpool.tile([BL, H], F16, tag="h0T", name="h0T")
    h0f = wpool.tile([1, BL * H], F16, tag="h0f", name="h0f")
    h0f_v = h0f[:].rearrange("o (b k) -> o b k", b=BL)
    # ones[0:255]=1, ones[255]=0: the bias matmul then leaves each batch's
    # pad column exactly 0 (xi pad col is 0 too), so tanh(0)=0 and the
    # reduce can cover the full 256-col block without masking.
    ones_sb = wpool.tile([1, NP], F16, tag="ones", name="ones")
    nc.gpsimd.memset(ones_sb[:], 1.0)
    nc.gpsimd.memset(ones_sb[0:1, NI:NP], 0.0)

    # PSUM: main pool 3 x [128,1024]f32 (2 banks each) for the wave tiles
    # and ph0; small pool 2 x 1 bank for warm-up + the two epilogue
    # accumulators. 3*2 + 2 = 8 banks.
    ppool = ctx.enter_context(tc.tile_pool(name="ps", bufs=3, space="PSUM"))
    spool = ctx.enter_context(tc.tile_pool(name="pss", bufs=2, space="PSUM"))
    itpool = ctx.enter_context(tc.tile_pool(name="it", bufs=6))

    # ---- Phase 0: preload the tanh ACT table during the DMA lead-in
    # (first ACTIVATE otherwise pays the ~1.3us table load mid-kernel).
    tiny = wpool.tile([128, 1], F32, tag="tiny", name="tiny")
    nc.gpsimd.memset(tiny[:], 0.0)
    nc.scalar.activation(tiny[:], tiny[:], Tanh)

    # ---- Phase 0b: PE warm-up while w1a streams (see N_WARM).
    wz = wpool.tile([128, 256], F32, tag="warmz", name="warmz")
    nc.gpsimd.memset(wz[:], 0.0)
    pw = spool.tile([128, 256], F32, tag="pss", name="pwarm")
    for _ in range(N_WARM):
        nc.tensor.matmul(pw[:], wz[:, :128], wz[:], start=True, stop=True)

    # ---- Phase 1 (batch-major, by H-half): h0T[b,h] = sum_f x0T[f,b]
    # W1aT[f,h]; each half is cast to fp16 by an ACT copy (32x psum scale
    # baked in) and flattened to partition 0 by a scalar-queue SBUF->SBUF
    # DMA, so per-(b,htile) rows become legal matmul lhsT slices (SBUF
    # access patterns may only start at partitions 0/32/64/96).
    ph0 = ppool.tile([BL, H], F32, tag="ps", name="ph0")
    for hh in range(2):
        sl = slice(hh * 512, (hh + 1) * 512)
        for f in range(NF):
            nc.tensor.matmul(
                ph0[:, sl],
                x0_sb[f],
                w1a_sb[hh][f][:],
                start=(f == 0),
                stop=(f == NF - 1),
            )
        nc.scalar.activation(h0T_sb[:, sl], ph0[:, sl], Copy, scale=W1B_SCALE)
        nc.scalar.dma_start(h0f_v[:, :, sl], h0T_sb[:, sl])

    def bias_mms(pb, h, q):
        # Four rank-1 fp16 matmuls add 32*h0[b, htile] to every column of
        # batch b's 256-col block. The PSUM start bit zeroes a whole 2KB
        # bank (512 f32 cols), so only the first matmul per bank sets it;
        # the second accumulates onto the bank's zeroed other half.
        for j in range(4):
            b = q * 4 + j
            nc.tensor.matmul(
                pb[:, j * NP : (j + 1) * NP],
                h0f[0:1, (b * NH + h) * 128 : (b * NH + h + 1) * 128],
                ones_sb[:],
                start=(j % 2 == 0),
                stop=False,
                skip_group_check=True,
            )

    def main_mms(pb, h, q):
        # fp8 DoubleRow: 2 ksub-pairs x 2 col-halves, each contracting 256
        # features over 512 psum columns.
        for bk in range(2):
            cols = slice(bk * 512, (bk + 1) * 512)
            for kp in range(2):
                nc.tensor.matmul(
                    pb[:, cols],
                    w1b_v[:, 2 * kp : 2 * kp + 2, h * 128 : (h + 1) * 128],
                    xi_sb[q][:, 2 * kp : 2 * kp + 2, cols],
                    start=False,
                    stop=(kp == 1),
                    perf_mode=DR,
                    skip_group_check=True,
                )

    def consume(h, q, pb, idx):
        # ACT: one big tanh over the whole psum tile (pad cols are exact
        # zeros). A halving add folds cols [128:256) onto [0:128), then a
        # half-size DVE segmented reduce. Pool (gpsimd) measures
        # ~2.25ns/elem on TENSOR_TENSOR vs DVE 2x_1p at ~0.5ns/elem, so
        # adds alternate between them; free-axis reduce is DVE-only.
        it = itpool.tile([128, 4 * NP], BF16, tag="it", name=f"it_{h}_{q}")
        nc.scalar.activation(it[:], pb[:], Tanh, scale=1.0 / W1B_SCALE)
        v = it[:].rearrange("p (b n) -> p b n", b=4)
        half = itpool.tile([128, 4 * 128], BF16, tag="half", name=f"hf_{h}_{q}")
        hv = half[:].rearrange("p (b n) -> p b n", b=4)
        add_eng = nc.gpsimd if idx % 2 == 0 else nc.vector
        with nc.allow_low_precision(
            reason="S accumulated in 16-bit to feed the fp16 epilogue matmul"
        ):
            add_eng.tensor_add(hv, v[:, :, 0:128], v[:, :, 128:256])
            nc.vector.reduce_sum(
                S_sb[h][:, q * 4 : (q + 1) * 4],
                hv,
                axis=mybir.AxisListType.X,
            )

    # ---- Phase 2. Tile order: h-pair-major, wave-inner, so each h's four
    # quads finish early and its W2 epilogue matmul can issue mid-stream.
    # The first N_PREBIAS tiles' bias matmuls go up front (they only need
    # h0), and N_WARM2 dummies bridge the PE gap until xi0 lands.
    order = []
    for hp in range(0, NH, 2):
        for w in range(2):
            for h in (hp, hp + 1):
                for q in (2 * w, 2 * w + 1):
                    order.append((h, q))
    done_count = {h: 0 for h in range(NH)}
    w2_pending = []
    po_issued = [0]
    po = spool.tile([BL, F], F32, tag="pss", name="po")

    def po_mm(lhsT, rhs):
        # One shared 12-matmul accumulation group: 8 W2 terms issued as
        # each S[h] completes mid-stream, 4 W0 terms slotted in at idx 17.
        nc.tensor.matmul(
            po[:], lhsT, rhs,
            start=(po_issued[0] == 0),
            stop=(po_issued[0] == NH + NF - 1),
            skip_group_check=True,
        )
        po_issued[0] += 1

    def flush_w2():
        h = w2_pending.pop(0)
        po_mm(S_sb[h][:], w2_sb[h][:])

    pbs = {}
    for h, q in order[:N_PREBIAS]:
        pb = pbs[(h, q)] = ppool.tile([128, 4 * NP], F32, tag="ps", name=f"pb_{h}_{q}")
        bias_mms(pb, h, q)
    for _ in range(N_WARM2):
        nc.tensor.matmul(pw[:], wz[:, :128], wz[:], start=True, stop=True)

    for idx, (h, q) in enumerate(order):
        if w2_pending and idx >= 2:
            flush_w2()
        if idx == 17:
            # W0 epilogue (independent of S): x0/w0 are long since loaded;
            # issuing mid-stream keeps it off the tail.
            for f in range(NF):
                po_mm(x0_sb[f], w0_sb[f][:])
        if (h, q) in pbs:
            pb = pbs.pop((h, q))
        else:
            pb = ppool.tile([128, 4 * NP], F32, tag="ps", name=f"pb_{h}_{q}")
            bias_mms(pb, h, q)
        main_mms(pb, h, q)
        consume(h, q, pb, idx)
        done_count[h] += 1
        if done_count[h] == 4:
            w2_pending.append(h)
    while w2_pending:
        flush_w2()

    # ---- Phase 3 tail: one copy from PSUM, then out.
    rt = itpool.tile([BL, F], F32, tag="rt", name="rt")
    nc.vector.tensor_copy(rt[:], po[:])
    nc.sync.dma_start(res[:], rt[:])


_NC_CACHE = {}


def _get_nc():
    key = ("v4", N_PREBIAS, N_WARM, N_WARM2)
    if key not in _NC_CACHE:
        _NC_CACHE[key] = _build_kernel()
    return _NC_CACHE[key]


def _make_in_maps(x, W1, W2, W0):
    import ml_dtypes
    f8 = ml_dtypes.float8_e4m3
    f16 = np.float16

    x = np.ascontiguousarray(np.asarray(x, dtype=np.float32))
    W1 = np.asarray(W1, dtype=np.float32)
    W2 = np.asarray(W2, dtype=np.float32)
    W0 = np.asarray(W0, dtype=np.float32)

    # [p, k, h] = 32 * W1b[h, k*128+p]
    w1bT = np.ascontiguousarray(
        (W1[:, F:].T * W1B_SCALE).reshape(NF, 128, H).transpose(1, 0, 2).reshape(128, NF * H)
    ).astype(f8)
    w1aT = np.ascontiguousarray(W1[:, :F].T).astype(f16)   # [F, H]
    w2T = np.ascontiguousarray(W2.T).astype(f16)           # [H, F]
    w0T = np.ascontiguousarray(W0.T).astype(f16)           # [F, F]

    in_maps = []
    for i in range(N_CORES):
        xc = x[i * BL : (i + 1) * BL]               # [BL, N, F]
        # packed [128, NF*BL]: row p, block f holds x0T[f*128+p, :]
        x0Tc = np.ascontiguousarray(
            xc[:, 0, :].T.reshape(NF, 128, BL).transpose(1, 0, 2).reshape(128, NF * BL)
        ).astype(f16)
        pad = np.zeros((BL, NP, F), dtype=np.float32)
        pad[:, :NI, :] = xc[:, 1:, :]
        xiT_full = pad.reshape(BL * NP, F).T        # [512, 4096], col = 256*b + n
        # [p, q, k, c] = xiT_full[k*128+p, q*1024+c]
        xiTc = np.ascontiguousarray(
            xiT_full.reshape(NF, 128, QUADS, 1024)
            .transpose(1, 2, 0, 3)
            .reshape(128, QUADS * NF * 1024)
        ).astype(f8)
        in_maps.append(
            {
                "xiT": xiTc,
                "x0T": x0Tc,
                "w1bT": w1bT,
                "w1aT": w1aT,
                "w2T": w2T,
                "w0T": w0T,
            }
        )
    return in_maps


def _gather(results):
    out = np.empty((B, F), dtype=np.float32)
    for i in range(N_CORES):
        out[i * BL : (i + 1) * BL] = results[i]["res"]
    return out


def kernel(x, W1, W2, W0):
    nc = _get_nc()
    in_maps = _make_in_maps(x, W1, W2, W0)
    res = run_bass_kernel_spmd(nc, in_maps, list(range(N_CORES)))
    return _gather(res.results)


def kernel_profiled(x, W1, W2, W0, **trace_kwargs):
    """Like kernel() but with NTFF profiling; returns (out, exec_time_ns)."""
    nc = _get_nc()
    in_maps = _make_in_maps(x, W1, W2, W0)
    res = run_bass_kernel_spmd(
        nc, in_maps, list(range(N_CORES)), trace=True, **trace_kwargs
    )
    return _gather(res.results), res.exec_time_ns
